# revision 30
# baseline (speedup 1.0000x reference)
"""Multi-head causal attention (B=2, S=2048, D=1024, H=16) on 8 NeuronCores.

Sharding: core c = (batch b=c//4, head-group g=c%4 of 4 heads).
Each core projects Q/K (transposed layout) and V for its 4 heads from the
host-transposed input xT, runs causal attention head-by-head over 512-query
windows in the transposed-score layout ST[k, q] (all matmul operands bf16,
softmax denominator fused into the A@V matmul via a ones-column in V).
Two 8-core AllToAlls (one per head pair; the first fires at the attention
midpoint and overlaps the second pair) swap head-shards for query-shards;
every core then runs the output projection on a fixed local 256-query slice
of each batch, overlapping the pair-0 accumulation with the second AllToAll.

PSUM budget (8 banks): tag A = 2x[128,1024] (scores / qk-proj chunks /
out-proj), tag pot = 2x[65,512] (attention output + fused denominator),
tag fill = 2x[128,512] (warmup dummies, V-proj).
"""

import numpy as np

import concourse.bass as bass
import concourse.mybir as mybir
import concourse.tile as tile
from concourse import bacc
from concourse.bass_utils import run_bass_kernel_spmd

B, S, D = 2, 2048, 1024
H = 16
DH = 64  # head dim
N_CORES = 8
GROUPS = 4  # cores per batch = head groups
H_LOC = H // GROUPS  # 4 heads per core
EH = H_LOC * DH  # 256 local qkv width
QCH = 512  # query chunk
NCH = S // QCH  # 4
KB = 128  # key block
NKB = S // KB  # 16
NDB = D // 128  # 8 contraction blocks
QL = 256  # local output query rows per batch
VW = DH + 1  # 65: V columns + fused ones column
SCALE = 1.0 / 8.0  # 1/sqrt(DH)

F32 = mybir.dt.float32
BF16 = mybir.dt.bfloat16
MM_DT = BF16
EXP = mybir.ActivationFunctionType.Exp
MULT = mybir.AluOpType.mult
ADD = mybir.AluOpType.add


def _emit(nc, tc, xT, wq_d, wk_d, wv_d, wo_d, bb_d, y_d):
    from contextlib import ExitStack

    ctx = ExitStack()
    with ctx:
        persist = ctx.enter_context(tc.tile_pool(name="persist", bufs=1))
        psum = ctx.enter_context(tc.tile_pool(name="psum", bufs=1, space="PSUM"))
        dram = ctx.enter_context(tc.tile_pool(name="dram", bufs=1, space="DRAM"))
        work = ctx.enter_context(tc.tile_pool(name="work", bufs=1))

        # Entry barrier: align all 8 cores before any work. Kernel launches
        # are staggered across cores; without this every collective absorbs
        # that skew as dead wait on the early cores. Modeled as a tiny
        # AllGather (so the Tile scheduler understands it) whose result gates
        # a corner-write of every input tile, ordering the input DMAs after it.
        bar_in = dram.tile([1, 4], F32, name="barin")
        bar_out = dram.tile([N_CORES, 4], F32, name="barout")
        nc.gpsimd.collective_compute(
            "AllGather",
            mybir.AluOpType.bypass,
            replica_groups=[list(range(N_CORES))],
            ins=[bar_in[:]],
            outs=[bar_out[:]],
        )
        gate1 = persist.tile([1, 4], F32)
        nc.sync.dma_start(gate1[:], bar_out[0:1, :])

        # --- constants ---
        # tri[k, t] = 1 if t >= k else 0 (bf16): causal mask for a diagonal
        # 128-key x 128-query sub-block.
        tri = persist.tile([128, 128], MM_DT)
        nc.gpsimd.memset(tri[:], 1.0)
        nc.gpsimd.affine_select(
            out=tri[:],
            in_=tri[:],
            compare_op=mybir.AluOpType.is_ge,
            fill=0.0,
            base=0,
            channel_multiplier=-1,
            pattern=[[1, 128]],
        )
        ones_f = persist.tile([128, 1], F32)
        nc.gpsimd.memset(ones_f[:], 1.0)
        bb_sb = persist.tile([128, D], F32)
        # dummy matmul source for PE warmup during the input-DMA phase
        dmy = persist.tile([128, 512], MM_DT)
        nc.gpsimd.memset(dmy[:], 0.0)

        _dmy_n = [0]

        def warm_burst(n):
            # keep the PE HAM clock-gate at full rate while real matmuls are
            # DMA-gated; results are discarded
            for _ in range(n):
                i = _dmy_n[0]
                _dmy_n[0] += 1
                ps = psum.tile([128, 512], F32, tag="fill", bufs=2, name=f"dmy{i}")
                nc.tensor.matmul(ps[:], dmy[:, 0:128], dmy[:], start=True, stop=True)

        # --- persistent operand tiles ---
        xt_sb = [persist.tile([128, S], MM_DT, name=f"xt{d}") for d in range(NDB)]
        w_sb = {
            nm: persist.tile([128, NDB * EH], MM_DT, name=f"w{nm}sb")
            for nm in ("q", "k", "v")
        }
        wo_sb = persist.tile([128, NDB * D], MM_DT)
        qt = [persist.tile([128, S], MM_DT, name=f"qt{p}") for p in range(2)]
        kt = [persist.tile([128, S], MM_DT, name=f"kt{p}") for p in range(2)]
        vg = [persist.tile([128, NKB * VW], MM_DT, name=f"vg{h}") for h in range(H_LOC)]
        for h in range(H_LOC):
            nc.vector.tensor_copy(
                vg[h].rearrange("p (n w) -> p n w", w=VW)[:, :, DH : DH + 1],
                ones_f[:].unsqueeze(2).broadcast_to([128, NKB, 1]),
            )
        oft_own = [persist.tile([128, S], MM_DT, name=f"oftown{p}") for p in range(2)]
        oft_all = [persist.tile([128, 2 * QL], MM_DT, name=f"oft{f}") for f in range(NDB)]

        # --- input DMAs (inputs are pre-cast to bf16 on the host) ---
        # corner-write every input tile from the barrier gate first: the real
        # DMA then has a WAW dependency on the barrier
        for t in xt_sb + [w_sb["q"], w_sb["k"], w_sb["v"], wo_sb]:
            nc.gpsimd.tensor_copy(t[0:1, 0:1], gate1[0:1, 0:1])
        nc.gpsimd.tensor_copy(bb_sb[0:1, 0:1], gate1[0:1, 0:1])
        # wq/wk first (gate the first projection matmuls), xT on the sync
        # queue concurrently, then wv, bias (wo streams during attention).
        for nm, wd in (("q", wq_d), ("k", wk_d)):
            nc.gpsimd.dma_start(
                w_sb[nm][:].rearrange("p (db e) -> p db e", db=NDB),
                wd.rearrange("(db p) e -> p db e", p=128),
            )
        for d in range(NDB):
            nc.sync.dma_start(xt_sb[d][:], xT[d * 128 : (d + 1) * 128, :])
        nc.gpsimd.dma_start(
            w_sb["v"][:].rearrange("p (db e) -> p db e", db=NDB),
            wv_d.rearrange("(db p) e -> p db e", p=128),
        )
        nc.gpsimd.dma_start(bb_sb[:], bb_d[:])

        # --- qk projection for pair p, query half jh (cols jh*1024..+1024) ---
        def emit_proj_qk_half(p, jh, warm=False):
            for dst, nm in ((qt[p], "q"), (kt[p], "k")):
                ps = psum.tile([128, 2 * QCH], F32, tag="A", bufs=2, name=f"pp{nm}{p}{jh}")
                for d in range(NDB):
                    for half in range(2):
                        nc.tensor.matmul(
                            ps[:, half * QCH : (half + 1) * QCH],
                            w_sb[nm][:, d * EH + 128 * p : d * EH + 128 * p + 128],
                            xt_sb[d][
                                :, jh * 2 * QCH + half * QCH : jh * 2 * QCH + (half + 1) * QCH
                            ],
                            start=(d == 0),
                            stop=(d == NDB - 1),
                        )
                    if warm:
                        warm_burst(2)
                nc.vector.tensor_copy(
                    dst[:, jh * 2 * QCH : (jh + 1) * 2 * QCH], ps[:]
                )

        # --- V projection for one 128-key block ---
        def emit_proj_v(sb_i):
            ps = psum.tile([128, EH], F32, tag="fill", bufs=2, name=f"pv{sb_i}")
            for d in range(NDB):
                nc.tensor.matmul(
                    ps[:],
                    xt_sb[d][:, sb_i * KB : (sb_i + 1) * KB],
                    w_sb["v"][:, d * EH : (d + 1) * EH],
                    start=(d == 0),
                    stop=(d == NDB - 1),
                )
            for h in range(H_LOC):
                nc.vector.tensor_copy(
                    vg[h][:, sb_i * VW : sb_i * VW + DH],
                    ps[:, h * DH : (h + 1) * DH],
                )

        # --- wo blocks (gpsimd queue, spread across attention) ---
        def emit_wo_block(f):
            nc.gpsimd.dma_start(
                wo_sb[:, f * D : (f + 1) * D], wo_d[f * 128 : (f + 1) * 128, :]
            )

        # --- attention for head h, query chunk j ---
        def emit_attn_chunk(h, j):
            p, r = h // 2, DH * (h % 2)
            pot = psum.tile([VW, QCH], F32, tag="pot", bufs=2, name=f"pot{h}_{j}")
            nkb_j = 4 * (j + 1)
            for g in range(2 * (j + 1)):
                if g % 2 == 0:
                    warm_burst(1)  # HAM keepalive: PE micro-gaps re-throttle the clock
                pss = psum.tile([128, 2 * QCH], F32, tag="A", bufs=2, name=f"ps{h}{j}{g}")
                c0s = []
                for sub in range(2):
                    kb = 2 * g + sub
                    c0 = max(0, 128 * kb - QCH * j)
                    c0s.append(c0)
                    nc.tensor.matmul(
                        pss[:, sub * QCH + c0 : (sub + 1) * QCH],
                        kt[p][r : r + DH, kb * KB : (kb + 1) * KB],
                        qt[p][r : r + DH, j * QCH + c0 : (j + 1) * QCH],
                        start=True,
                        stop=True,
                    )
                e = work.tile([128, 2 * QCH], MM_DT, tag="e", bufs=3, name=f"e{h}{j}{g}")
                nc.scalar.activation(
                    e[:, c0s[0] :], pss[:, c0s[0] :], EXP, scale=SCALE
                )
                for sub in range(2):
                    kb = 2 * g + sub
                    m = kb - 4 * j
                    if 0 <= m <= 3:  # diagonal sub-block: zero the triangle
                        ct = sub * QCH + 128 * m
                        nc.vector.tensor_tensor(
                            e[:, ct : ct + 128], e[:, ct : ct + 128], tri[:], op=MULT
                        )
                for sub in range(2):
                    kb = 2 * g + sub
                    c0 = c0s[sub]
                    nc.tensor.matmul(
                        pot[:, c0:QCH],
                        vg[h][:, kb * VW : (kb + 1) * VW],
                        e[:, sub * QCH + c0 : (sub + 1) * QCH],
                        start=(kb == 0),
                        stop=(kb == nkb_j - 1),
                    )
            # normalize: oft_own = pot[0:64] / pot[64] (softmax denominator)
            den = work.tile([1, QCH], F32, tag="den", bufs=2, name=f"den{h}_{j}")
            nc.vector.tensor_copy(den[:], pot[DH : DH + 1, :])
            rec = work.tile([1, QCH], F32, tag="rec", bufs=2, name=f"rec{h}_{j}")
            nc.vector.reciprocal_approx_fast(rec[:], den[:])
            pb = work.tile([DH, QCH], F32, tag="pb", bufs=2, name=f"pb{h}_{j}")
            nc.gpsimd.partition_broadcast(pb[:], rec[0:1, :])
            nc.vector.tensor_tensor(
                oft_own[p][r : r + DH, j * QCH : (j + 1) * QCH],
                pot[0:DH, :],
                pb[:],
                op=MULT,
            )

        # --- A2A plumbing ---
        a2a_bufs = {}

        def emit_a2a_cin(p, j):
            if p not in a2a_bufs:
                cin = dram.tile([N_CORES * 128, QL], MM_DT, name=f"cin{p}")
                cout = dram.tile([N_CORES * 128, QL], MM_DT, name=f"cout{p}")
                a2a_bufs[p] = (cin, cout)
            cin = a2a_bufs[p][0]
            for s in (2 * j, 2 * j + 1):
                nc.sync.dma_start(
                    cin[s * 128 : (s + 1) * 128, :],
                    oft_own[p][:, s * QL : (s + 1) * QL],
                )

        def emit_a2a_trigger(p):
            cin, cout = a2a_bufs[p]
            nc.gpsimd.collective_compute(
                "AllToAll",
                mybir.AluOpType.bypass,
                replica_groups=[list(range(N_CORES))],
                ins=[cin[:]],
                outs=[cout[:]],
            )

        def emit_a2a_post(p):
            cin, cout = a2a_bufs[p]
            for rr in range(GROUPS):
                for bi in range(2):
                    src_rank = bi * GROUPS + rr
                    nc.sync.dma_start(
                        oft_all[2 * rr + p][:, bi * QL : (bi + 1) * QL],
                        cout[src_rank * 128 : (src_rank + 1) * 128, :],
                    )

        # ===== emission schedule =====
        warm_burst(16)
        emit_proj_qk_half(0, 0, warm=True)
        for sb_i in range(4):
            warm_burst(4)
            emit_proj_v(sb_i)

        # h=0: interleave remaining V-proj blocks + second qk half between
        # query chunks (chunks 0/1 only touch qt/kt cols [0, 1024))
        emit_attn_chunk(0, 0)
        for sb_i in range(4, 8):
            emit_proj_v(sb_i)
        emit_attn_chunk(0, 1)
        for sb_i in range(8, 12):
            emit_proj_v(sb_i)
        emit_proj_qk_half(0, 1)
        emit_attn_chunk(0, 2)
        for sb_i in range(12, 16):
            emit_proj_v(sb_i)
        emit_attn_chunk(0, 3)

        # h=1: descending query chunks (the last chunk before the A2A trigger
        # is the smallest), interleave pair-1 qk projection + first wo blocks
        emit_attn_chunk(1, 3)
        emit_a2a_cin(0, 3)
        emit_proj_qk_half(1, 0)
        emit_attn_chunk(1, 2)
        emit_a2a_cin(0, 2)
        emit_wo_block(0)
        emit_wo_block(1)
        emit_attn_chunk(1, 1)
        emit_a2a_cin(0, 1)
        emit_proj_qk_half(1, 1)
        emit_attn_chunk(1, 0)
        emit_a2a_cin(0, 0)
        emit_a2a_trigger(0)

        emit_attn_chunk(2, 0)
        emit_wo_block(2)
        emit_wo_block(3)
        emit_attn_chunk(2, 1)
        emit_wo_block(4)
        emit_wo_block(5)
        emit_attn_chunk(2, 2)
        emit_wo_block(6)
        emit_wo_block(7)
        emit_attn_chunk(2, 3)

        emit_attn_chunk(3, 3)
        emit_a2a_cin(1, 3)
        emit_attn_chunk(3, 2)
        emit_a2a_cin(1, 2)
        emit_attn_chunk(3, 1)
        emit_a2a_cin(1, 1)
        emit_attn_chunk(3, 0)
        emit_a2a_cin(1, 0)
        emit_a2a_post(0)
        emit_a2a_trigger(1)
        emit_a2a_post(1)

        # --- output projection on local 256-query slice of each batch ---
        # pair-0 f-blocks (evens) for ALL output tiles first: they only need
        # A2A#0 and overlap the A2A#1 wait; odds accumulate when A2A#1 lands.
        # 4 output tiles need all 8 banks simultaneously (held from the even
        # accumulation through the odd one); spread them over the three tags'
        # slots, which the attention loop has released by now.
        pys = {
            (0, 0): [(psum.tile([128, D], F32, tag="A", bufs=2, name="py00"), 0)],
            (0, 1): [(psum.tile([128, D], F32, tag="A", bufs=2, name="py01"), 0)],
            (1, 0): [
                (psum.tile([128, QCH], F32, tag="pot", bufs=2, name="py10a"), 0),
                (psum.tile([128, QCH], F32, tag="pot", bufs=2, name="py10b"), QCH),
            ],
            (1, 1): [
                (psum.tile([128, QCH], F32, tag="fill", bufs=2, name="py11a"), 0),
                (psum.tile([128, QCH], F32, tag="fill", bufs=2, name="py11b"), QCH),
            ],
        }

        def py_slice(key, ech):
            parts = pys[key]
            if len(parts) == 1:
                return parts[0][0][:, ech * QCH : (ech + 1) * QCH]
            return parts[ech][0][:, 0:QCH]

        # Gate matmuls: zero contribution (moving operand is the zeros tile),
        # but the stationary operand reads oft_own[1] cols 0:128 — written by
        # the LAST attention normalize (h3, chunk 0). This is a deliberate
        # fence: the PE queue is in-order, so without it the scheduler hoists
        # these out-proj accumulations (which wait on the collective's DMAs)
        # into the attention stream and stalls attention behind the A2A.
        for bi in range(2):
            for qb in range(QL // 128):
                for ech in range(2):
                    nc.tensor.matmul(
                        py_slice((bi, qb), ech),
                        oft_own[1][:, 0:128],
                        dmy[:],
                        start=True,
                        stop=False,
                    )
        for phase, fs in enumerate(([0, 2, 4, 6], [1, 3, 5, 7])):
            for bi in range(2):
                for qb in range(QL // 128):
                    for fi, f in enumerate(fs):
                        for ech in range(2):
                            nc.tensor.matmul(
                                py_slice((bi, qb), ech),
                                oft_all[f][
                                    :, bi * QL + qb * 128 : bi * QL + (qb + 1) * 128
                                ],
                                wo_sb[:, f * D + ech * QCH : f * D + ech * QCH + QCH],
                                start=False,
                                stop=(phase == 1 and fi == 3),
                            )
        for bi in range(2):
            for qb in range(QL // 128):
                ysb = work.tile([128, D], F32, tag="ysb", bufs=2, name=f"y{bi}_{qb}")
                for ech in range(2):
                    nc.vector.tensor_tensor(
                        ysb[:, ech * QCH : (ech + 1) * QCH],
                        py_slice((bi, qb), ech),
                        bb_sb[:, ech * QCH : (ech + 1) * QCH],
                        op=ADD,
                    )
                nc.sync.dma_start(
                    y_d[bi * QL + qb * 128 : bi * QL + (qb + 1) * 128, :], ysb[:]
                )


def build_program():
    nc = bacc.Bacc(
        "TRN2", target_bir_lowering=False, debug=False, num_devices=N_CORES
    )
    xT = nc.dram_tensor("xT", [D, S], BF16, kind="ExternalInput")
    wq = nc.dram_tensor("wq", [D, EH], BF16, kind="ExternalInput")
    wk = nc.dram_tensor("wk", [D, EH], BF16, kind="ExternalInput")
    wv = nc.dram_tensor("wv", [D, EH], BF16, kind="ExternalInput")
    wo = nc.dram_tensor("wo", [D, D], BF16, kind="ExternalInput")
    bb = nc.dram_tensor("bb", [128, D], F32, kind="ExternalInput")
    y = nc.dram_tensor("y", [2 * QL, D], F32, kind="ExternalOutput")
    with tile.TileContext(nc) as tc:
        _emit(nc, tc, xT.ap(), wq.ap(), wk.ap(), wv.ap(), wo.ap(), bb.ap(), y.ap())
    nc.compile()
    return nc


_cached_nc = None


def _get_nc():
    global _cached_nc
    if _cached_nc is None:
        _cached_nc = build_program()
    return _cached_nc


def make_in_maps(x, w_qkv, w_out, b_out):
    import ml_dtypes

    bf16 = ml_dtypes.bfloat16
    x = np.asarray(x, np.float32).astype(bf16)
    w_qkv = np.asarray(w_qkv, np.float32).astype(bf16)
    w_out = np.ascontiguousarray(np.asarray(w_out, np.float32).astype(bf16))
    b_out = np.asarray(b_out, np.float32)
    bb = np.ascontiguousarray(np.broadcast_to(b_out, (128, D)))
    in_maps = []
    for c in range(N_CORES):
        b, g = c // GROUPS, c % GROUPS
        in_maps.append(
            {
                "xT": np.ascontiguousarray(x[b].T),
                "wq": np.ascontiguousarray(w_qkv[:, g * EH : (g + 1) * EH]),
                "wk": np.ascontiguousarray(w_qkv[:, D + g * EH : D + (g + 1) * EH]),
                "wv": np.ascontiguousarray(
                    w_qkv[:, 2 * D + g * EH : 2 * D + (g + 1) * EH]
                ),
                "wo": w_out,
                "bb": bb,
            }
        )
    return in_maps


def assemble(results):
    # core c's y is [512, D]: rows [0,256) = batch 0 q-slice [256c, 256c+256),
    # rows [256,512) = batch 1 same slice.
    y = np.empty((B, S, D), np.float32)
    for c in range(N_CORES):
        yc = results[c]["y"]
        y[0, 256 * c : 256 * (c + 1), :] = yc[:256]
        y[1, 256 * c : 256 * (c + 1), :] = yc[256:]
    return y


def kernel(x, w_qkv, w_out, b_out, _trace=False, **run_kwargs):
    nc = _get_nc()
    in_maps = make_in_maps(x, w_qkv, w_out, b_out)
    res = run_bass_kernel_spmd(
        nc, in_maps, core_ids=list(range(N_CORES)), trace=_trace, **run_kwargs
    )
    out = assemble(res.results)
    if _trace:
        return out, res
    return out


# revision 33
# speedup vs baseline: 1.1858x; 1.1858x over previous
"""Multi-head causal attention (B=2, S=2048, D=1024, H=16) on 8 NeuronCores.

Sharding: core c = (batch b=c//4, head-group g=c%4 of 4 heads).
Each core projects Q/K (transposed layout) and V for its 4 heads from the
host-transposed input xT, runs causal attention head-by-head over 512-query
windows in the transposed-score layout ST[k, q] (all matmul operands bf16,
softmax denominator fused into the A@V matmul via a ones-column in V).
Two 8-core AllToAlls (one per head pair; the first fires at the attention
midpoint and overlaps the second pair) swap head-shards for query-shards;
every core then runs the output projection on a fixed local 256-query slice
of each batch, overlapping the pair-0 accumulation with the second AllToAll.

PSUM budget (8 banks): tag A = 2x[128,1024] (scores / qk-proj chunks /
out-proj), tag pot = 2x[65,512] (attention output + fused denominator),
tag fill = 2x[128,512] (warmup dummies, V-proj).
"""

import numpy as np

import concourse.bass as bass
import concourse.mybir as mybir
import concourse.tile as tile
from concourse import bacc
from concourse.bass_utils import run_bass_kernel_spmd

B, S, D = 2, 2048, 1024
H = 16
DH = 64  # head dim
N_CORES = 8
GROUPS = 4  # cores per batch = head groups
H_LOC = H // GROUPS  # 4 heads per core
EH = H_LOC * DH  # 256 local qkv width
QCH = 512  # query chunk
NCH = S // QCH  # 4
KB = 128  # key block
NKB = S // KB  # 16
NDB = D // 128  # 8 contraction blocks
QL = 256  # local output query rows per batch
VW = DH + 1  # 65: V columns + fused ones column
SCALE = 1.0 / 8.0  # 1/sqrt(DH)

F32 = mybir.dt.float32
BF16 = mybir.dt.bfloat16
MM_DT = BF16
EXP = mybir.ActivationFunctionType.Exp
MULT = mybir.AluOpType.mult
ADD = mybir.AluOpType.add


def _emit(nc, tc, xT, wq_d, wk_d, wv_d, wo_d, bb_d, y_d):
    from contextlib import ExitStack

    ctx = ExitStack()
    with ctx:
        persist = ctx.enter_context(tc.tile_pool(name="persist", bufs=1))
        psum = ctx.enter_context(tc.tile_pool(name="psum", bufs=1, space="PSUM"))
        dram = ctx.enter_context(tc.tile_pool(name="dram", bufs=1, space="DRAM"))
        work = ctx.enter_context(tc.tile_pool(name="work", bufs=1))

        # --- constants ---
        # tri[k, t] = 1 if t >= k else 0 (bf16): causal mask for a diagonal
        # 128-key x 128-query sub-block.
        tri = persist.tile([128, 128], MM_DT)
        nc.gpsimd.memset(tri[:], 1.0)
        nc.gpsimd.affine_select(
            out=tri[:],
            in_=tri[:],
            compare_op=mybir.AluOpType.is_ge,
            fill=0.0,
            base=0,
            channel_multiplier=-1,
            pattern=[[1, 128]],
        )
        ones_f = persist.tile([128, 1], F32)
        nc.gpsimd.memset(ones_f[:], 1.0)
        bb_sb = persist.tile([128, D], F32)
        # dummy matmul source for PE warmup during the input-DMA phase
        dmy = persist.tile([128, 512], MM_DT)
        nc.gpsimd.memset(dmy[:], 0.0)

        _dmy_n = [0]

        def warm_burst(n):
            # keep the PE HAM clock-gate at full rate while real matmuls are
            # DMA-gated; results are discarded
            for _ in range(n):
                i = _dmy_n[0]
                _dmy_n[0] += 1
                ps = psum.tile([128, 512], F32, tag="fill", bufs=2, name=f"dmy{i}")
                nc.tensor.matmul(ps[:], dmy[:, 0:128], dmy[:], start=True, stop=True)

        # --- persistent operand tiles ---
        xt_sb = [persist.tile([128, S], MM_DT, name=f"xt{d}") for d in range(NDB)]
        w_sb = {
            nm: persist.tile([128, NDB * EH], MM_DT, name=f"w{nm}sb")
            for nm in ("q", "k", "v")
        }
        wo_sb = persist.tile([128, NDB * D], MM_DT)
        qt = [persist.tile([128, S], MM_DT, name=f"qt{p}") for p in range(2)]
        kt = [persist.tile([128, S], MM_DT, name=f"kt{p}") for p in range(2)]
        vg = [persist.tile([128, NKB * VW], MM_DT, name=f"vg{h}") for h in range(H_LOC)]
        for h in range(H_LOC):
            nc.vector.tensor_copy(
                vg[h].rearrange("p (n w) -> p n w", w=VW)[:, :, DH : DH + 1],
                ones_f[:].unsqueeze(2).broadcast_to([128, NKB, 1]),
            )
        oft_own = [persist.tile([128, S], MM_DT, name=f"oftown{p}") for p in range(2)]
        oft_all = [persist.tile([128, 2 * QL], MM_DT, name=f"oft{f}") for f in range(NDB)]

        # --- input DMAs (inputs are pre-cast to bf16 on the host) ---
        # wq/wk first (gate the first projection matmuls), xT split over the
        # sync and vector queues concurrently, then wv, bias (wo streams
        # during attention).
        for nm, wd in (("q", wq_d), ("k", wk_d)):
            nc.gpsimd.dma_start(
                w_sb[nm][:].rearrange("p (db e) -> p db e", db=NDB),
                wd.rearrange("(db p) e -> p db e", p=128),
            )
        for d in range(NDB):
            eng = nc.sync if d % 2 == 0 else nc.scalar
            eng.dma_start(xt_sb[d][:], xT[d * 128 : (d + 1) * 128, :])
        nc.gpsimd.dma_start(
            w_sb["v"][:].rearrange("p (db e) -> p db e", db=NDB),
            wv_d.rearrange("(db p) e -> p db e", p=128),
        )
        nc.gpsimd.dma_start(bb_sb[:], bb_d[:])

        # --- qk projection for pair p, query half jh (cols jh*1024..+1024) ---
        def emit_proj_qk_half(p, jh, warm=False):
            for dst, nm in ((qt[p], "q"), (kt[p], "k")):
                ps = psum.tile([128, 2 * QCH], F32, tag="A", bufs=2, name=f"pp{nm}{p}{jh}")
                for d in range(NDB):
                    for half in range(2):
                        nc.tensor.matmul(
                            ps[:, half * QCH : (half + 1) * QCH],
                            w_sb[nm][:, d * EH + 128 * p : d * EH + 128 * p + 128],
                            xt_sb[d][
                                :, jh * 2 * QCH + half * QCH : jh * 2 * QCH + (half + 1) * QCH
                            ],
                            start=(d == 0),
                            stop=(d == NDB - 1),
                        )
                    if warm:
                        warm_burst(2)
                nc.vector.tensor_copy(
                    dst[:, jh * 2 * QCH : (jh + 1) * 2 * QCH], ps[:]
                )

        # --- V projection for one 128-key block ---
        def emit_proj_v(sb_i):
            ps = psum.tile([128, EH], F32, tag="fill", bufs=2, name=f"pv{sb_i}")
            for d in range(NDB):
                nc.tensor.matmul(
                    ps[:],
                    xt_sb[d][:, sb_i * KB : (sb_i + 1) * KB],
                    w_sb["v"][:, d * EH : (d + 1) * EH],
                    start=(d == 0),
                    stop=(d == NDB - 1),
                )
            for h in range(H_LOC):
                nc.vector.tensor_copy(
                    vg[h][:, sb_i * VW : sb_i * VW + DH],
                    ps[:, h * DH : (h + 1) * DH],
                )

        # --- wo blocks (gpsimd queue, spread across attention) ---
        def emit_wo_block(f):
            nc.gpsimd.dma_start(
                wo_sb[:, f * D : (f + 1) * D], wo_d[f * 128 : (f + 1) * 128, :]
            )

        # --- attention for head h, query chunk j ---
        def emit_attn_chunk(h, j):
            p, r = h // 2, DH * (h % 2)
            pot = psum.tile([VW, QCH], F32, tag="pot", bufs=2, name=f"pot{h}_{j}")
            nkb_j = 4 * (j + 1)
            for g in range(2 * (j + 1)):
                if g % 2 == 0:
                    warm_burst(1)  # HAM keepalive: PE micro-gaps re-throttle the clock
                pss = psum.tile([128, 2 * QCH], F32, tag="A", bufs=2, name=f"ps{h}{j}{g}")
                c0s = []
                for sub in range(2):
                    kb = 2 * g + sub
                    c0 = max(0, 128 * kb - QCH * j)
                    c0s.append(c0)
                    nc.tensor.matmul(
                        pss[:, sub * QCH + c0 : (sub + 1) * QCH],
                        kt[p][r : r + DH, kb * KB : (kb + 1) * KB],
                        qt[p][r : r + DH, j * QCH + c0 : (j + 1) * QCH],
                        start=True,
                        stop=True,
                    )
                e = work.tile([128, 2 * QCH], MM_DT, tag="e", bufs=3, name=f"e{h}{j}{g}")
                nc.scalar.activation(
                    e[:, c0s[0] :], pss[:, c0s[0] :], EXP, scale=SCALE
                )
                for sub in range(2):
                    kb = 2 * g + sub
                    m = kb - 4 * j
                    if 0 <= m <= 3:  # diagonal sub-block: zero the triangle
                        ct = sub * QCH + 128 * m
                        nc.vector.tensor_tensor(
                            e[:, ct : ct + 128], e[:, ct : ct + 128], tri[:], op=MULT
                        )
                for sub in range(2):
                    kb = 2 * g + sub
                    c0 = c0s[sub]
                    nc.tensor.matmul(
                        pot[:, c0:QCH],
                        vg[h][:, kb * VW : (kb + 1) * VW],
                        e[:, sub * QCH + c0 : (sub + 1) * QCH],
                        start=(kb == 0),
                        stop=(kb == nkb_j - 1),
                    )
            # normalize: oft_own = pot[0:64] / pot[64] (softmax denominator)
            den = work.tile([1, QCH], F32, tag="den", bufs=2, name=f"den{h}_{j}")
            nc.vector.tensor_copy(den[:], pot[DH : DH + 1, :])
            rec = work.tile([1, QCH], F32, tag="rec", bufs=2, name=f"rec{h}_{j}")
            nc.vector.reciprocal_approx_fast(rec[:], den[:])
            pb = work.tile([DH, QCH], F32, tag="pb", bufs=2, name=f"pb{h}_{j}")
            nc.gpsimd.partition_broadcast(pb[:], rec[0:1, :])
            nc.vector.tensor_tensor(
                oft_own[p][r : r + DH, j * QCH : (j + 1) * QCH],
                pot[0:DH, :],
                pb[:],
                op=MULT,
            )

        # --- A2A plumbing ---
        a2a_bufs = {}

        def emit_a2a_cin(p, j):
            if p not in a2a_bufs:
                cin = dram.tile([N_CORES * 128, QL], MM_DT, name=f"cin{p}")
                cout = dram.tile([N_CORES * 128, QL], MM_DT, name=f"cout{p}")
                a2a_bufs[p] = (cin, cout)
            cin = a2a_bufs[p][0]
            for s in (2 * j, 2 * j + 1):
                nc.sync.dma_start(
                    cin[s * 128 : (s + 1) * 128, :],
                    oft_own[p][:, s * QL : (s + 1) * QL],
                )

        def emit_a2a_trigger(p):
            cin, cout = a2a_bufs[p]
            nc.gpsimd.collective_compute(
                "AllToAll",
                mybir.AluOpType.bypass,
                replica_groups=[list(range(N_CORES))],
                ins=[cin[:]],
                outs=[cout[:]],
            )

        def emit_a2a_post(p):
            cin, cout = a2a_bufs[p]
            for rr in range(GROUPS):
                for bi in range(2):
                    src_rank = bi * GROUPS + rr
                    nc.sync.dma_start(
                        oft_all[2 * rr + p][:, bi * QL : (bi + 1) * QL],
                        cout[src_rank * 128 : (src_rank + 1) * 128, :],
                    )

        # ===== emission schedule =====
        warm_burst(16)
        emit_proj_qk_half(0, 0, warm=True)
        for sb_i in range(4):
            warm_burst(4)
            emit_proj_v(sb_i)

        # h=0: interleave remaining V-proj blocks + second qk half between
        # query chunks (chunks 0/1 only touch qt/kt cols [0, 1024))
        emit_attn_chunk(0, 0)
        for sb_i in range(4, 8):
            emit_proj_v(sb_i)
        emit_attn_chunk(0, 1)
        for sb_i in range(8, 12):
            emit_proj_v(sb_i)
        emit_proj_qk_half(0, 1)
        emit_attn_chunk(0, 2)
        for sb_i in range(12, 16):
            emit_proj_v(sb_i)
        emit_attn_chunk(0, 3)

        # h=1: descending query chunks (the last chunk before the A2A trigger
        # is the smallest), interleave pair-1 qk projection + first wo blocks
        emit_attn_chunk(1, 3)
        emit_a2a_cin(0, 3)
        emit_proj_qk_half(1, 0)
        emit_attn_chunk(1, 2)
        emit_a2a_cin(0, 2)
        emit_wo_block(0)
        emit_wo_block(1)
        emit_attn_chunk(1, 1)
        emit_a2a_cin(0, 1)
        emit_proj_qk_half(1, 1)
        emit_attn_chunk(1, 0)
        emit_a2a_cin(0, 0)
        emit_a2a_trigger(0)

        emit_attn_chunk(2, 0)
        emit_wo_block(2)
        emit_wo_block(3)
        emit_attn_chunk(2, 1)
        emit_wo_block(4)
        emit_wo_block(5)
        emit_attn_chunk(2, 2)
        emit_wo_block(6)
        emit_wo_block(7)
        emit_attn_chunk(2, 3)

        emit_attn_chunk(3, 3)
        emit_a2a_cin(1, 3)
        emit_attn_chunk(3, 2)
        emit_a2a_cin(1, 2)
        emit_attn_chunk(3, 1)
        emit_a2a_cin(1, 1)
        emit_attn_chunk(3, 0)
        emit_a2a_cin(1, 0)
        emit_a2a_post(0)
        emit_a2a_trigger(1)
        emit_a2a_post(1)

        # --- output projection on local 256-query slice of each batch ---
        # pair-0 f-blocks (evens) for ALL output tiles first: they only need
        # A2A#0 and overlap the A2A#1 wait; odds accumulate when A2A#1 lands.
        # 4 output tiles need all 8 banks simultaneously (held from the even
        # accumulation through the odd one); spread them over the three tags'
        # slots, which the attention loop has released by now.
        pys = {
            (0, 0): [(psum.tile([128, D], F32, tag="A", bufs=2, name="py00"), 0)],
            (0, 1): [(psum.tile([128, D], F32, tag="A", bufs=2, name="py01"), 0)],
            (1, 0): [
                (psum.tile([128, QCH], F32, tag="pot", bufs=2, name="py10a"), 0),
                (psum.tile([128, QCH], F32, tag="pot", bufs=2, name="py10b"), QCH),
            ],
            (1, 1): [
                (psum.tile([128, QCH], F32, tag="fill", bufs=2, name="py11a"), 0),
                (psum.tile([128, QCH], F32, tag="fill", bufs=2, name="py11b"), QCH),
            ],
        }

        def py_slice(key, ech):
            parts = pys[key]
            if len(parts) == 1:
                return parts[0][0][:, ech * QCH : (ech + 1) * QCH]
            return parts[ech][0][:, 0:QCH]

        # Gate matmuls: zero contribution (moving operand is the zeros tile),
        # but the stationary operand reads oft_own[1] cols 0:128 — written by
        # the LAST attention normalize (h3, chunk 0). This is a deliberate
        # fence: the PE queue is in-order, so without it the scheduler hoists
        # these out-proj accumulations (which wait on the collective's DMAs)
        # into the attention stream and stalls attention behind the A2A.
        for bi in range(2):
            for qb in range(QL // 128):
                for ech in range(2):
                    nc.tensor.matmul(
                        py_slice((bi, qb), ech),
                        oft_own[1][:, 0:128],
                        dmy[:],
                        start=True,
                        stop=False,
                    )
        for phase, fs in enumerate(([0, 2, 4, 6], [1, 3, 5, 7])):
            for bi in range(2):
                for qb in range(QL // 128):
                    for fi, f in enumerate(fs):
                        for ech in range(2):
                            nc.tensor.matmul(
                                py_slice((bi, qb), ech),
                                oft_all[f][
                                    :, bi * QL + qb * 128 : bi * QL + (qb + 1) * 128
                                ],
                                wo_sb[:, f * D + ech * QCH : f * D + ech * QCH + QCH],
                                start=False,
                                stop=(phase == 1 and fi == 3),
                            )
        for bi in range(2):
            for qb in range(QL // 128):
                ysb = work.tile([128, D], F32, tag="ysb", bufs=2, name=f"y{bi}_{qb}")
                for ech in range(2):
                    nc.vector.tensor_tensor(
                        ysb[:, ech * QCH : (ech + 1) * QCH],
                        py_slice((bi, qb), ech),
                        bb_sb[:, ech * QCH : (ech + 1) * QCH],
                        op=ADD,
                    )
                nc.sync.dma_start(
                    y_d[bi * QL + qb * 128 : bi * QL + (qb + 1) * 128, :], ysb[:]
                )


def build_program():
    nc = bacc.Bacc(
        "TRN2", target_bir_lowering=False, debug=False, num_devices=N_CORES
    )
    xT = nc.dram_tensor("xT", [D, S], BF16, kind="ExternalInput")
    wq = nc.dram_tensor("wq", [D, EH], BF16, kind="ExternalInput")
    wk = nc.dram_tensor("wk", [D, EH], BF16, kind="ExternalInput")
    wv = nc.dram_tensor("wv", [D, EH], BF16, kind="ExternalInput")
    wo = nc.dram_tensor("wo", [D, D], BF16, kind="ExternalInput")
    bb = nc.dram_tensor("bb", [128, D], F32, kind="ExternalInput")
    y = nc.dram_tensor("y", [2 * QL, D], F32, kind="ExternalOutput")
    with tile.TileContext(nc) as tc:
        _emit(nc, tc, xT.ap(), wq.ap(), wk.ap(), wv.ap(), wo.ap(), bb.ap(), y.ap())
    nc.compile()
    return nc


_cached_nc = None


def _get_nc():
    global _cached_nc
    if _cached_nc is None:
        _cached_nc = build_program()
    return _cached_nc


def make_in_maps(x, w_qkv, w_out, b_out):
    import ml_dtypes

    bf16 = ml_dtypes.bfloat16
    x = np.asarray(x, np.float32).astype(bf16)
    w_qkv = np.asarray(w_qkv, np.float32).astype(bf16)
    w_out = np.ascontiguousarray(np.asarray(w_out, np.float32).astype(bf16))
    b_out = np.asarray(b_out, np.float32)
    bb = np.ascontiguousarray(np.broadcast_to(b_out, (128, D)))
    in_maps = []
    for c in range(N_CORES):
        b, g = c // GROUPS, c % GROUPS
        in_maps.append(
            {
                "xT": np.ascontiguousarray(x[b].T),
                "wq": np.ascontiguousarray(w_qkv[:, g * EH : (g + 1) * EH]),
                "wk": np.ascontiguousarray(w_qkv[:, D + g * EH : D + (g + 1) * EH]),
                "wv": np.ascontiguousarray(
                    w_qkv[:, 2 * D + g * EH : 2 * D + (g + 1) * EH]
                ),
                "wo": w_out,
                "bb": bb,
            }
        )
    return in_maps


def assemble(results):
    # core c's y is [512, D]: rows [0,256) = batch 0 q-slice [256c, 256c+256),
    # rows [256,512) = batch 1 same slice.
    y = np.empty((B, S, D), np.float32)
    for c in range(N_CORES):
        yc = results[c]["y"]
        y[0, 256 * c : 256 * (c + 1), :] = yc[:256]
        y[1, 256 * c : 256 * (c + 1), :] = yc[256:]
    return y


def kernel(x, w_qkv, w_out, b_out, _trace=False, **run_kwargs):
    nc = _get_nc()
    in_maps = make_in_maps(x, w_qkv, w_out, b_out)
    res = run_bass_kernel_spmd(
        nc, in_maps, core_ids=list(range(N_CORES)), trace=_trace, **run_kwargs
    )
    out = assemble(res.results)
    if _trace:
        return out, res
    return out


# revision 34
# speedup vs baseline: 1.2533x; 1.0569x over previous
"""Multi-head causal attention (B=2, S=2048, D=1024, H=16) on 8 NeuronCores.

Sharding: core c = (batch b=c//4, head-group g=c%4 of 4 heads).
Each core projects Q/K (transposed layout) and V for its 4 heads from the
host-transposed input xT, runs causal attention head-by-head over 512-query
windows in the transposed-score layout ST[k, q] (all matmul operands bf16,
softmax denominator fused into the A@V matmul via a ones-column in V).
Two 8-core AllToAlls (one per head pair; the first fires at the attention
midpoint and overlaps the second pair) swap head-shards for query-shards;
every core then runs the output projection on a fixed local 256-query slice
of each batch, overlapping the pair-0 accumulation with the second AllToAll.

PSUM budget (8 banks): tag A = 2x[128,1024] (scores / qk-proj chunks /
out-proj), tag pot = 2x[65,512] (attention output + fused denominator),
tag fill = 2x[128,512] (warmup dummies, V-proj).
"""

import numpy as np

import concourse.bass as bass
import concourse.mybir as mybir
import concourse.tile as tile
from concourse import bacc
from concourse.bass_utils import run_bass_kernel_spmd

B, S, D = 2, 2048, 1024
H = 16
DH = 64  # head dim
N_CORES = 8
GROUPS = 4  # cores per batch = head groups
H_LOC = H // GROUPS  # 4 heads per core
EH = H_LOC * DH  # 256 local qkv width
QCH = 512  # query chunk
NCH = S // QCH  # 4
KB = 128  # key block
NKB = S // KB  # 16
NDB = D // 128  # 8 contraction blocks
QL = 256  # local output query rows per batch
VW = DH + 1  # 65: V columns + fused ones column
SCALE = 1.0 / 8.0  # 1/sqrt(DH)

F32 = mybir.dt.float32
BF16 = mybir.dt.bfloat16
MM_DT = BF16
EXP = mybir.ActivationFunctionType.Exp
MULT = mybir.AluOpType.mult
ADD = mybir.AluOpType.add


def _emit(nc, tc, xT, wq_d, wk_d, wv_d, wo_d, bb_d, y_d):
    from contextlib import ExitStack

    ctx = ExitStack()
    with ctx:
        persist = ctx.enter_context(tc.tile_pool(name="persist", bufs=1))
        psum = ctx.enter_context(tc.tile_pool(name="psum", bufs=1, space="PSUM"))
        dram = ctx.enter_context(tc.tile_pool(name="dram", bufs=1, space="DRAM"))
        work = ctx.enter_context(tc.tile_pool(name="work", bufs=1))

        # --- constants ---
        # tri[k, t] = 1 if t >= k else 0 (bf16): causal mask for a diagonal
        # 128-key x 128-query sub-block.
        tri = persist.tile([128, 128], MM_DT)
        nc.gpsimd.memset(tri[:], 1.0)
        nc.gpsimd.affine_select(
            out=tri[:],
            in_=tri[:],
            compare_op=mybir.AluOpType.is_ge,
            fill=0.0,
            base=0,
            channel_multiplier=-1,
            pattern=[[1, 128]],
        )
        ones_f = persist.tile([128, 1], F32)
        nc.gpsimd.memset(ones_f[:], 1.0)
        bb_sb = persist.tile([128, D], F32)
        # dummy matmul source for PE warmup during the input-DMA phase
        dmy = persist.tile([128, 512], MM_DT)
        nc.gpsimd.memset(dmy[:], 0.0)

        _dmy_n = [0]

        def warm_burst(n):
            # keep the PE HAM clock-gate at full rate while real matmuls are
            # DMA-gated; results are discarded
            for _ in range(n):
                i = _dmy_n[0]
                _dmy_n[0] += 1
                ps = psum.tile([128, 512], F32, tag="fill", bufs=2, name=f"dmy{i}")
                nc.tensor.matmul(ps[:], dmy[:, 0:128], dmy[:], start=True, stop=True)

        # --- persistent operand tiles ---
        xt_sb = [persist.tile([128, S], MM_DT, name=f"xt{d}") for d in range(NDB)]
        w_sb = {
            nm: persist.tile([128, NDB * EH], MM_DT, name=f"w{nm}sb")
            for nm in ("q", "k", "v")
        }
        wo_sb = persist.tile([128, NDB * D], MM_DT)
        qt = [persist.tile([128, S], MM_DT, name=f"qt{p}") for p in range(2)]
        kt = [persist.tile([128, S], MM_DT, name=f"kt{p}") for p in range(2)]
        vg = [persist.tile([128, NKB * VW], MM_DT, name=f"vg{h}") for h in range(H_LOC)]
        for h in range(H_LOC):
            nc.vector.tensor_copy(
                vg[h].rearrange("p (n w) -> p n w", w=VW)[:, :, DH : DH + 1],
                ones_f[:].unsqueeze(2).broadcast_to([128, NKB, 1]),
            )
        oft_own = [persist.tile([128, S], MM_DT, name=f"oftown{p}") for p in range(2)]
        oft_all = [persist.tile([128, 2 * QL], MM_DT, name=f"oft{f}") for f in range(NDB)]

        # --- input DMAs (inputs are pre-cast to bf16 on the host) ---
        # wq/wk first (gate the first projection matmuls), xT split over the
        # sync and vector queues concurrently, then wv, bias (wo streams
        # during attention).
        for nm, wd in (("q", wq_d), ("k", wk_d)):
            nc.gpsimd.dma_start(
                w_sb[nm][:].rearrange("p (db e) -> p db e", db=NDB),
                wd.rearrange("(db p) e -> p db e", p=128),
            )
        for d in range(NDB):
            eng = nc.sync if d % 2 == 0 else nc.scalar
            eng.dma_start(xt_sb[d][:], xT[d * 128 : (d + 1) * 128, :])
        nc.gpsimd.dma_start(
            w_sb["v"][:].rearrange("p (db e) -> p db e", db=NDB),
            wv_d.rearrange("(db p) e -> p db e", p=128),
        )
        nc.gpsimd.dma_start(bb_sb[:], bb_d[:])

        # --- qk projection for pair p, query half jh (cols jh*1024..+1024) ---
        def emit_proj_qk_half(p, jh, warm=False):
            for dst, nm in ((qt[p], "q"), (kt[p], "k")):
                ps = psum.tile([128, 2 * QCH], F32, tag="A", bufs=2, name=f"pp{nm}{p}{jh}")
                for d in range(NDB):
                    for half in range(2):
                        nc.tensor.matmul(
                            ps[:, half * QCH : (half + 1) * QCH],
                            w_sb[nm][:, d * EH + 128 * p : d * EH + 128 * p + 128],
                            xt_sb[d][
                                :, jh * 2 * QCH + half * QCH : jh * 2 * QCH + (half + 1) * QCH
                            ],
                            start=(d == 0),
                            stop=(d == NDB - 1),
                        )
                    if warm:
                        warm_burst(2)
                nc.vector.tensor_copy(
                    dst[:, jh * 2 * QCH : (jh + 1) * 2 * QCH], ps[:]
                )

        # --- V projection for one 128-key block ---
        def emit_proj_v(sb_i):
            ps = psum.tile([128, EH], F32, tag="fill", bufs=2, name=f"pv{sb_i}")
            for d in range(NDB):
                nc.tensor.matmul(
                    ps[:],
                    xt_sb[d][:, sb_i * KB : (sb_i + 1) * KB],
                    w_sb["v"][:, d * EH : (d + 1) * EH],
                    start=(d == 0),
                    stop=(d == NDB - 1),
                )
            for h in range(H_LOC):
                nc.vector.tensor_copy(
                    vg[h][:, sb_i * VW : sb_i * VW + DH],
                    ps[:, h * DH : (h + 1) * DH],
                )

        # --- wo blocks (gpsimd queue, spread across attention) ---
        def emit_wo_block(f):
            nc.gpsimd.dma_start(
                wo_sb[:, f * D : (f + 1) * D], wo_d[f * 128 : (f + 1) * 128, :]
            )

        # --- attention for head h, query chunk j ---
        def emit_attn_chunk(h, j):
            p, r = h // 2, DH * (h % 2)
            pot = psum.tile([VW, QCH], F32, tag="pot", bufs=2, name=f"pot{h}_{j}")
            nkb_j = 4 * (j + 1)
            for g in range(2 * (j + 1)):
                pss = psum.tile([128, 2 * QCH], F32, tag="A", bufs=2, name=f"ps{h}{j}{g}")
                c0s = []
                for sub in range(2):
                    kb = 2 * g + sub
                    c0 = max(0, 128 * kb - QCH * j)
                    c0s.append(c0)
                    nc.tensor.matmul(
                        pss[:, sub * QCH + c0 : (sub + 1) * QCH],
                        kt[p][r : r + DH, kb * KB : (kb + 1) * KB],
                        qt[p][r : r + DH, j * QCH + c0 : (j + 1) * QCH],
                        start=True,
                        stop=True,
                    )
                e = work.tile([128, 2 * QCH], MM_DT, tag="e", bufs=3, name=f"e{h}{j}{g}")
                nc.scalar.activation(
                    e[:, c0s[0] :], pss[:, c0s[0] :], EXP, scale=SCALE
                )
                for sub in range(2):
                    kb = 2 * g + sub
                    m = kb - 4 * j
                    if 0 <= m <= 3:  # diagonal sub-block: zero the triangle
                        ct = sub * QCH + 128 * m
                        nc.vector.tensor_tensor(
                            e[:, ct : ct + 128], e[:, ct : ct + 128], tri[:], op=MULT
                        )
                for sub in range(2):
                    kb = 2 * g + sub
                    c0 = c0s[sub]
                    nc.tensor.matmul(
                        pot[:, c0:QCH],
                        vg[h][:, kb * VW : (kb + 1) * VW],
                        e[:, sub * QCH + c0 : (sub + 1) * QCH],
                        start=(kb == 0),
                        stop=(kb == nkb_j - 1),
                    )
            # normalize: oft_own = pot[0:64] / pot[64] (softmax denominator)
            den = work.tile([1, QCH], F32, tag="den", bufs=2, name=f"den{h}_{j}")
            nc.vector.tensor_copy(den[:], pot[DH : DH + 1, :])
            rec = work.tile([1, QCH], F32, tag="rec", bufs=2, name=f"rec{h}_{j}")
            nc.vector.reciprocal_approx_fast(rec[:], den[:])
            pb = work.tile([DH, QCH], F32, tag="pb", bufs=2, name=f"pb{h}_{j}")
            nc.gpsimd.partition_broadcast(pb[:], rec[0:1, :])
            nc.vector.tensor_tensor(
                oft_own[p][r : r + DH, j * QCH : (j + 1) * QCH],
                pot[0:DH, :],
                pb[:],
                op=MULT,
            )

        # --- A2A plumbing ---
        a2a_bufs = {}

        def emit_a2a_cin(p, j):
            if p not in a2a_bufs:
                cin = dram.tile([N_CORES * 128, QL], MM_DT, name=f"cin{p}")
                cout = dram.tile([N_CORES * 128, QL], MM_DT, name=f"cout{p}")
                a2a_bufs[p] = (cin, cout)
            cin = a2a_bufs[p][0]
            for s in (2 * j, 2 * j + 1):
                nc.sync.dma_start(
                    cin[s * 128 : (s + 1) * 128, :],
                    oft_own[p][:, s * QL : (s + 1) * QL],
                )

        def emit_a2a_trigger(p):
            cin, cout = a2a_bufs[p]
            nc.gpsimd.collective_compute(
                "AllToAll",
                mybir.AluOpType.bypass,
                replica_groups=[list(range(N_CORES))],
                ins=[cin[:]],
                outs=[cout[:]],
            )

        def emit_a2a_post(p):
            cin, cout = a2a_bufs[p]
            for rr in range(GROUPS):
                for bi in range(2):
                    src_rank = bi * GROUPS + rr
                    nc.sync.dma_start(
                        oft_all[2 * rr + p][:, bi * QL : (bi + 1) * QL],
                        cout[src_rank * 128 : (src_rank + 1) * 128, :],
                    )

        # ===== emission schedule =====
        warm_burst(16)
        emit_proj_qk_half(0, 0, warm=True)
        for sb_i in range(4):
            warm_burst(4)
            emit_proj_v(sb_i)

        # h=0: interleave remaining V-proj blocks + second qk half between
        # query chunks (chunks 0/1 only touch qt/kt cols [0, 1024))
        emit_attn_chunk(0, 0)
        for sb_i in range(4, 8):
            emit_proj_v(sb_i)
        emit_attn_chunk(0, 1)
        for sb_i in range(8, 12):
            emit_proj_v(sb_i)
        emit_proj_qk_half(0, 1)
        emit_attn_chunk(0, 2)
        for sb_i in range(12, 16):
            emit_proj_v(sb_i)
        emit_attn_chunk(0, 3)

        # h=1: descending query chunks (the last chunk before the A2A trigger
        # is the smallest), interleave pair-1 qk projection + first wo blocks
        emit_attn_chunk(1, 3)
        emit_a2a_cin(0, 3)
        emit_proj_qk_half(1, 0)
        emit_attn_chunk(1, 2)
        emit_a2a_cin(0, 2)
        emit_wo_block(0)
        emit_wo_block(1)
        emit_attn_chunk(1, 1)
        emit_a2a_cin(0, 1)
        emit_proj_qk_half(1, 1)
        emit_attn_chunk(1, 0)
        emit_a2a_cin(0, 0)
        emit_a2a_trigger(0)

        emit_attn_chunk(2, 0)
        emit_wo_block(2)
        emit_wo_block(3)
        emit_attn_chunk(2, 1)
        emit_wo_block(4)
        emit_wo_block(5)
        emit_attn_chunk(2, 2)
        emit_wo_block(6)
        emit_wo_block(7)
        emit_attn_chunk(2, 3)

        emit_attn_chunk(3, 3)
        emit_a2a_cin(1, 3)
        emit_attn_chunk(3, 2)
        emit_a2a_cin(1, 2)
        emit_attn_chunk(3, 1)
        emit_a2a_cin(1, 1)
        emit_attn_chunk(3, 0)
        emit_a2a_cin(1, 0)
        emit_a2a_post(0)
        emit_a2a_trigger(1)
        emit_a2a_post(1)

        # --- output projection on local 256-query slice of each batch ---
        # pair-0 f-blocks (evens) for ALL output tiles first: they only need
        # A2A#0 and overlap the A2A#1 wait; odds accumulate when A2A#1 lands.
        # 4 output tiles need all 8 banks simultaneously (held from the even
        # accumulation through the odd one); spread them over the three tags'
        # slots, which the attention loop has released by now.
        pys = {
            (0, 0): [(psum.tile([128, D], F32, tag="A", bufs=2, name="py00"), 0)],
            (0, 1): [(psum.tile([128, D], F32, tag="A", bufs=2, name="py01"), 0)],
            (1, 0): [
                (psum.tile([128, QCH], F32, tag="pot", bufs=2, name="py10a"), 0),
                (psum.tile([128, QCH], F32, tag="pot", bufs=2, name="py10b"), QCH),
            ],
            (1, 1): [
                (psum.tile([128, QCH], F32, tag="fill", bufs=2, name="py11a"), 0),
                (psum.tile([128, QCH], F32, tag="fill", bufs=2, name="py11b"), QCH),
            ],
        }

        def py_slice(key, ech):
            parts = pys[key]
            if len(parts) == 1:
                return parts[0][0][:, ech * QCH : (ech + 1) * QCH]
            return parts[ech][0][:, 0:QCH]

        # Gate matmuls: zero contribution (moving operand is the zeros tile),
        # but the stationary operand reads oft_own[1] cols 0:128 — written by
        # the LAST attention normalize (h3, chunk 0). This is a deliberate
        # fence: the PE queue is in-order, so without it the scheduler hoists
        # these out-proj accumulations (which wait on the collective's DMAs)
        # into the attention stream and stalls attention behind the A2A.
        for bi in range(2):
            for qb in range(QL // 128):
                for ech in range(2):
                    nc.tensor.matmul(
                        py_slice((bi, qb), ech),
                        oft_own[1][:, 0:128],
                        dmy[:],
                        start=True,
                        stop=False,
                    )
        for phase, fs in enumerate(([0, 2, 4, 6], [1, 3, 5, 7])):
            for bi in range(2):
                for qb in range(QL // 128):
                    for fi, f in enumerate(fs):
                        for ech in range(2):
                            nc.tensor.matmul(
                                py_slice((bi, qb), ech),
                                oft_all[f][
                                    :, bi * QL + qb * 128 : bi * QL + (qb + 1) * 128
                                ],
                                wo_sb[:, f * D + ech * QCH : f * D + ech * QCH + QCH],
                                start=False,
                                stop=(phase == 1 and fi == 3),
                            )
        for bi in range(2):
            for qb in range(QL // 128):
                ysb = work.tile([128, D], F32, tag="ysb", bufs=2, name=f"y{bi}_{qb}")
                for ech in range(2):
                    nc.vector.tensor_tensor(
                        ysb[:, ech * QCH : (ech + 1) * QCH],
                        py_slice((bi, qb), ech),
                        bb_sb[:, ech * QCH : (ech + 1) * QCH],
                        op=ADD,
                    )
                nc.sync.dma_start(
                    y_d[bi * QL + qb * 128 : bi * QL + (qb + 1) * 128, :], ysb[:]
                )


def build_program():
    nc = bacc.Bacc(
        "TRN2", target_bir_lowering=False, debug=False, num_devices=N_CORES
    )
    xT = nc.dram_tensor("xT", [D, S], BF16, kind="ExternalInput")
    wq = nc.dram_tensor("wq", [D, EH], BF16, kind="ExternalInput")
    wk = nc.dram_tensor("wk", [D, EH], BF16, kind="ExternalInput")
    wv = nc.dram_tensor("wv", [D, EH], BF16, kind="ExternalInput")
    wo = nc.dram_tensor("wo", [D, D], BF16, kind="ExternalInput")
    bb = nc.dram_tensor("bb", [128, D], F32, kind="ExternalInput")
    y = nc.dram_tensor("y", [2 * QL, D], F32, kind="ExternalOutput")
    with tile.TileContext(nc) as tc:
        _emit(nc, tc, xT.ap(), wq.ap(), wk.ap(), wv.ap(), wo.ap(), bb.ap(), y.ap())
    nc.compile()
    return nc


_cached_nc = None


def _get_nc():
    global _cached_nc
    if _cached_nc is None:
        _cached_nc = build_program()
    return _cached_nc


def make_in_maps(x, w_qkv, w_out, b_out):
    import ml_dtypes

    bf16 = ml_dtypes.bfloat16
    x = np.asarray(x, np.float32).astype(bf16)
    w_qkv = np.asarray(w_qkv, np.float32).astype(bf16)
    w_out = np.ascontiguousarray(np.asarray(w_out, np.float32).astype(bf16))
    b_out = np.asarray(b_out, np.float32)
    bb = np.ascontiguousarray(np.broadcast_to(b_out, (128, D)))
    in_maps = []
    for c in range(N_CORES):
        b, g = c // GROUPS, c % GROUPS
        in_maps.append(
            {
                "xT": np.ascontiguousarray(x[b].T),
                "wq": np.ascontiguousarray(w_qkv[:, g * EH : (g + 1) * EH]),
                "wk": np.ascontiguousarray(w_qkv[:, D + g * EH : D + (g + 1) * EH]),
                "wv": np.ascontiguousarray(
                    w_qkv[:, 2 * D + g * EH : 2 * D + (g + 1) * EH]
                ),
                "wo": w_out,
                "bb": bb,
            }
        )
    return in_maps


def assemble(results):
    # core c's y is [512, D]: rows [0,256) = batch 0 q-slice [256c, 256c+256),
    # rows [256,512) = batch 1 same slice.
    y = np.empty((B, S, D), np.float32)
    for c in range(N_CORES):
        yc = results[c]["y"]
        y[0, 256 * c : 256 * (c + 1), :] = yc[:256]
        y[1, 256 * c : 256 * (c + 1), :] = yc[256:]
    return y


def kernel(x, w_qkv, w_out, b_out, _trace=False, **run_kwargs):
    nc = _get_nc()
    in_maps = make_in_maps(x, w_qkv, w_out, b_out)
    res = run_bass_kernel_spmd(
        nc, in_maps, core_ids=list(range(N_CORES)), trace=_trace, **run_kwargs
    )
    out = assemble(res.results)
    if _trace:
        return out, res
    return out
